# revision 1
# baseline (speedup 1.0000x reference)
"""HGRN2Block kernel for 8 TRN2 NeuronCores.

Live path of the reference (the recurrence is dead code):
    x_proj = x @ W_proj + b_proj            # [B,L,3D]
    gate, _, ogate = split(x_proj, 3)       # middle third is DEAD
    out = (gate) * sigmoid(ogate)           # [B,L,D]
    out = out @ W_out + b_out               # [B,L,D]

Strategy:
  - Data-parallel over B*L rows: 16384 rows -> 2048 rows/core, no collectives.
  - Feature-major layout on device: host transposes x shard -> xT [D, rows]
    (free), so every matmul contracts over the SBUF partition dim without any
    on-device transpose. Output comes back as yT [D, rows]; host transposes.
  - bf16 matmuls (PSUM accumulates fp32): 4x the fp32 TensorE throughput.
  - Only the live 2/3 of W_proj is computed (cols [0:D] and [2D:3D]).
"""

import os

import numpy as np
import ml_dtypes

try:
    import concourse.bass as bass
except ImportError:
    import sys

    sys.path.insert(0, "/opt/trn_rl_repo")
    import concourse.bass as bass

import concourse.mybir as mybir
from concourse import bacc
from concourse.tile import TileContext
from concourse.bass_utils import run_bass_kernel_spmd

BF16 = ml_dtypes.bfloat16

B, L, D = 4, 4096, 1024
NCORES = 8
ROWS = B * L            # 16384
RPC = ROWS // NCORES    # 2048 rows per core
RB = 512                # moving free-dim per matmul (= one fp32 PSUM bank)
NRB = RPC // RB         # 4 row blocks per core
P = 128                 # SBUF partitions
KT = D // P             # 8 contraction tiles

_NC = None
LAST_RESULT = None      # BassKernelResults of the most recent run (for test.py)


def _build():
    nc = bacc.Bacc(trn_type="TRN2")
    f32 = mybir.dt.float32
    bf16 = mybir.dt.bfloat16

    xT = nc.dram_tensor("xT", [D, RPC], bf16, kind="ExternalInput")
    wg = nc.dram_tensor("wg", [D, D], bf16, kind="ExternalInput")
    wo = nc.dram_tensor("wo", [D, D], bf16, kind="ExternalInput")
    wout = nc.dram_tensor("wout", [D, D], bf16, kind="ExternalInput")
    bg = nc.dram_tensor("bg", [D], f32, kind="ExternalInput")
    bo = nc.dram_tensor("bo", [D], f32, kind="ExternalInput")
    bout = nc.dram_tensor("bout", [D], f32, kind="ExternalInput")
    yT = nc.dram_tensor("yT", [D, RPC], f32, kind="ExternalOutput")

    with TileContext(nc) as tc:
        with (
            tc.tile_pool(name="const", bufs=1) as cpool,
            tc.tile_pool(name="work", bufs=2) as wpool,
            tc.tile_pool(name="outp", bufs=4) as opool,
            tc.tile_pool(name="ps", bufs=2, space="PSUM") as pspool,
        ):
            # Biases: [D] -> SBUF [128, KT]; column k holds features k*128..
            bgS = cpool.tile([P, KT], f32, tag="bg", name="bgS")
            boS = cpool.tile([P, KT], f32, tag="bo", name="boS")
            boutS = cpool.tile([P, KT], f32, tag="bout", name="boutS")
            nc.gpsimd.dma_start(out=bgS, in_=bg[:].rearrange("(k p) -> p k", p=P))
            nc.gpsimd.dma_start(out=boS, in_=bo[:].rearrange("(k p) -> p k", p=P))
            nc.gpsimd.dma_start(out=boutS, in_=bout[:].rearrange("(k p) -> p k", p=P))

            # Warm-up: HAM starts the PE clock-gated at 1.2 GHz and only
            # ungates after ~3.4us of sustained activity. Spin matmuls on a
            # zeroed tile (no DMA deps) so the PE is warm before real work.
            wz = cpool.tile([P, RB], bf16, tag="wz", name="wz")
            nc.vector.memset(wz, 0.0)
            spin = pspool.tile([P, RB], f32, tag="spin", name="spin", bufs=1)
            for _ in range(28):
                nc.tensor.matmul(spin, lhsT=wz[:, :P], rhs=wz, start=True, stop=True)

            # x (feature-major) and all three weight matrices, resident in
            # SBUF. DMA order = need order: wg + rb0 columns of x first (the
            # first PSUM group needs all 8 k-tiles of wg), wout last.
            xS = [cpool.tile([P, RPC], bf16, tag=f"x{k}", name=f"xS{k}") for k in range(KT)]
            wgS = [cpool.tile([P, D], bf16, tag=f"wg{k}", name=f"wgS{k}") for k in range(KT)]
            woS = [cpool.tile([P, D], bf16, tag=f"wo{k}", name=f"woS{k}") for k in range(KT)]
            woutS = [cpool.tile([P, D], bf16, tag=f"wu{k}", name=f"wuS{k}") for k in range(KT)]
            # Queue roles: gpsimd carries weights (need order wo, wg, wout),
            # sync carries activations then shares with outputs, scalar's
            # queue stays empty so sigmoids never queue behind DMA triggers.
            for k in range(KT):
                r = slice(k * P, (k + 1) * P)
                nc.gpsimd.dma_start(out=woS[k], in_=wo[r, :])
                nc.sync.dma_start(out=xS[k][:, 0:RB], in_=xT[r, 0:RB])
            for k in range(KT):
                nc.gpsimd.dma_start(out=wgS[k], in_=wg[slice(k * P, (k + 1) * P), :])
            for rb in range(1, NRB):
                c = slice(rb * RB, (rb + 1) * RB)
                for k in range(KT):
                    nc.sync.dma_start(out=xS[k][:, c], in_=xT[slice(k * P, (k + 1) * P), c])
            for k in range(KT):
                nc.gpsimd.dma_start(out=woutS[k], in_=wout[slice(k * P, (k + 1) * P), :])

            for rb in range(NRB):
                cols = slice(rb * RB, (rb + 1) * RB)
                # ---- layer 1: hT/oT tiles -> gT = (hT+bg) * sigmoid(oT+bo)
                gS = [wpool.tile([P, RB], bf16, tag=f"g{m}", name=f"gS{rb}_{m}") for m in range(KT)]
                for m in range(KT):
                    msl = slice(m * P, (m + 1) * P)
                    ph = pspool.tile([P, RB], f32, tag="ph", name=f"ph{rb}_{m}")
                    po = pspool.tile([P, RB], f32, tag="po", name=f"po{rb}_{m}")
                    # o-group first: its sigmoid (ScalarE) overlaps the h-group
                    for k in range(KT):
                        nc.tensor.matmul(
                            po, lhsT=woS[k][:, msl], rhs=xS[k][:, cols],
                            start=(k == 0), stop=(k == KT - 1),
                        )
                    for k in range(KT):
                        nc.tensor.matmul(
                            ph, lhsT=wgS[k][:, msl], rhs=xS[k][:, cols],
                            start=(k == 0), stop=(k == KT - 1),
                        )
                    sig = opool.tile([P, RB], bf16, tag="sig", name=f"sig{rb}_{m}")
                    nc.scalar.activation(
                        out=sig, in_=po,
                        func=mybir.ActivationFunctionType.Sigmoid,
                        bias=boS[:, m : m + 1], scale=1.0,
                    )
                    nc.vector.scalar_tensor_tensor(
                        out=gS[m], in0=ph, scalar=bgS[:, m : m + 1], in1=sig,
                        op0=mybir.AluOpType.add, op1=mybir.AluOpType.mult,
                    )
                # ---- layer 2: yT = gT.T-contract @ W_out (+ b_out)
                for n in range(KT):
                    nsl = slice(n * P, (n + 1) * P)
                    py = pspool.tile([P, RB], f32, tag="py", name=f"py{rb}_{n}")
                    for m in range(KT):
                        nc.tensor.matmul(
                            py, lhsT=woutS[m][:, nsl], rhs=gS[m],
                            start=(m == 0), stop=(m == KT - 1),
                        )
                    yo = opool.tile([P, RB], f32, tag="yo", name=f"yo{rb}_{n}")
                    # bias-add on DVE, keeping ScalarE free for sigmoids
                    nc.vector.tensor_scalar_add(yo, py, boutS[:, n : n + 1])
                    nc.sync.dma_start(out=yT[nsl, cols], in_=yo)
    nc.finalize()
    return nc


def kernel(x, W_proj, b_proj, W_out, b_out, layer_idx=0, num_layers=12):
    global _NC, LAST_RESULT
    x = np.asarray(x, dtype=np.float32)
    W_proj = np.asarray(W_proj, dtype=np.float32)
    b_proj = np.asarray(b_proj, dtype=np.float32)
    W_out = np.asarray(W_out, dtype=np.float32)
    b_out = np.asarray(b_out, dtype=np.float32)

    wg = W_proj[:, :D].astype(BF16)
    wo = W_proj[:, 2 * D : 3 * D].astype(BF16)
    wu = W_out.astype(BF16)
    bg = np.ascontiguousarray(b_proj[:D])
    bo = np.ascontiguousarray(b_proj[2 * D : 3 * D])
    bu = np.ascontiguousarray(b_out)

    xf = x.reshape(ROWS, D)
    in_maps = []
    for c in range(NCORES):
        xs = xf[c * RPC : (c + 1) * RPC, :]
        xT = xs.T.astype(BF16)  # astype copies -> C-contiguous [D, RPC]
        in_maps.append(
            {"xT": xT, "wg": wg, "wo": wo, "wout": wu,
             "bg": bg, "bo": bo, "bout": bu}
        )

    if _NC is None:
        _NC = _build()

    trace = os.environ.get("HGRN_TRACE", "0") == "1"
    LAST_RESULT = run_bass_kernel_spmd(
        _NC, in_maps, core_ids=list(range(NCORES)), trace=trace,
        tmpdir=os.environ.get("HGRN_TMPDIR"),
    )
    y = np.empty((ROWS, D), dtype=np.float32)
    for c in range(NCORES):
        y[c * RPC : (c + 1) * RPC, :] = np.asarray(
            LAST_RESULT.results[c]["yT"], dtype=np.float32
        ).T
    return y.reshape(B, L, D)



# revision 7
# speedup vs baseline: 1.1308x; 1.1308x over previous
"""HGRN2Block kernel for 8 TRN2 NeuronCores.

Live path of the reference (the recurrence is dead code):
    x_proj = x @ W_proj + b_proj            # [B,L,3D]
    gate, _, ogate = split(x_proj, 3)       # middle third is DEAD
    out = (gate) * sigmoid(ogate)           # [B,L,D]
    out = out @ W_out + b_out               # [B,L,D]

Strategy:
  - Data-parallel over B*L rows: 16384 rows -> 2048 rows/core, no collectives.
  - Feature-major layout on device: host transposes x shard -> xT [D, rows]
    so every matmul contracts over the SBUF partition dim.
  - g-path and output matmuls in bf16 (PSUM accumulates fp32).
  - o-gate matmul in fp8e4m3 with DoubleRow (2 contraction rows per PE cell,
    ~1.8x bf16 MM rate). The sigmoid damps the fp8 quantization noise ~2.5x,
    so only this matmul tolerates fp8 within the error budget. Weights are
    pre-scaled x16 into fp8 range; the 1/16 folds into the activation scale.
  - k-tile-major DRAM layouts so each prologue weight/x transfer is one
    large dma_start (SWDGE/HWDGE fixed cost ~0.6-2us each), spread across
    the scalar/sync/gpsimd queues so the PE never starves at startup.
"""

import os

import numpy as np
import ml_dtypes

try:
    import concourse.bass as bass
except ImportError:
    import sys

    sys.path.insert(0, "/opt/trn_rl_repo")
    import concourse.bass as bass

import concourse.mybir as mybir
from concourse import bacc
from concourse.tile import TileContext
from concourse.bass_utils import run_bass_kernel_spmd

BF16 = ml_dtypes.bfloat16
F8 = ml_dtypes.float8_e4m3

B, L, D = 4, 4096, 1024
NCORES = 8
ROWS = B * L            # 16384
RPC = ROWS // NCORES    # 2048 rows per core
RB = 512                # moving free-dim per matmul (= one fp32 PSUM bank)
NRB = RPC // RB         # 4 row blocks per core
P = 128                 # SBUF partitions
KT = D // P             # 8 contraction tiles
KK = KT // 2            # 4 DoubleRow contraction pair-tiles
WO_SCALE = 16.0         # o-gate weights pre-scaled into fp8 range

_NC = None
LAST_RESULT = None      # BassKernelResults of the most recent run (for test.py)


def _build():
    nc = bacc.Bacc(trn_type="TRN2")
    f32 = mybir.dt.float32
    bf16 = mybir.dt.bfloat16
    fp8 = mybir.dt.float8e4
    DR = mybir.MatmulPerfMode.DoubleRow

    xb = nc.dram_tensor("xb", [KT, P, RPC], bf16, kind="ExternalInput")
    x8 = nc.dram_tensor("x8", [NRB, KK, P, 2, RB], fp8, kind="ExternalInput")
    wg = nc.dram_tensor("wg", [KT, P, D], bf16, kind="ExternalInput")
    wu = nc.dram_tensor("wu", [KT, P, D], bf16, kind="ExternalInput")
    wo8 = nc.dram_tensor("wo8", [KK, P, 2, D], fp8, kind="ExternalInput")
    bias = nc.dram_tensor("bias", [3, P, KT], f32, kind="ExternalInput")
    yT = nc.dram_tensor("yT", [D, RPC], bf16, kind="ExternalOutput")

    with TileContext(nc) as tc:
        with (
            tc.tile_pool(name="const", bufs=1) as cpool,
            tc.tile_pool(name="work", bufs=2) as wpool,
            tc.tile_pool(name="outp", bufs=4) as opool,
            tc.tile_pool(name="ps", bufs=2, space="PSUM") as pspool,
        ):
            biasS = cpool.tile([P, 3, KT], f32, tag="bias", name="biasS")
            wo8S = cpool.tile([P, KK, 2, D], fp8, tag="wo8", name="wo8S")
            x8S = cpool.tile([P, NRB, KK, 2, RB], fp8, tag="x8", name="x8S")
            xbS = cpool.tile([P, KT, RPC], bf16, tag="xb", name="xbS")
            wgS = cpool.tile([P, KT, D], bf16, tag="wg", name="wgS")
            wuS = cpool.tile([P, KT, D], bf16, tag="wu", name="wuS")

            # Prologue DMAs, one large transfer each, in need order per queue.
            # scalar (HWDGE): o-gate weights + biases -- feeds the first MMs
            nc.scalar.dma_start(
                out=wo8S[:, 0:2], in_=wo8[0:2].rearrange("k p t d -> p k t d")
            )
            nc.scalar.dma_start(out=biasS, in_=bias.rearrange("j p k -> p j k"))
            nc.scalar.dma_start(
                out=wo8S[:, 2:4], in_=wo8[2:4].rearrange("k p t d -> p k t d")
            )
            # sync (HWDGE): activations, first row block first
            nc.sync.dma_start(
                out=x8S[:, 0], in_=x8[0].rearrange("k p t c -> p k t c")
            )
            nc.sync.dma_start(
                out=xbS[:, :, 0:RB],
                in_=xb[:, :, 0:RB].rearrange("k p c -> p k c"),
            )
            for rb in range(1, NRB):
                nc.sync.dma_start(
                    out=x8S[:, rb], in_=x8[rb].rearrange("k p t c -> p k t c")
                )
            nc.sync.dma_start(
                out=xbS[:, :, RB:],
                in_=xb[:, :, RB:].rearrange("k p c -> p k c"),
            )
            # gpsimd (SWDGE): g-path weights, then output weights
            nc.gpsimd.dma_start(
                out=wgS[:, 0:4], in_=wg[0:4].rearrange("k p d -> p k d")
            )
            nc.gpsimd.dma_start(
                out=wgS[:, 4:8], in_=wg[4:8].rearrange("k p d -> p k d")
            )
            nc.gpsimd.dma_start(
                out=wuS[:, 0:4], in_=wu[0:4].rearrange("k p d -> p k d")
            )
            nc.gpsimd.dma_start(
                out=wuS[:, 4:8], in_=wu[4:8].rearrange("k p d -> p k d")
            )

            # Warm-up: HAM ungates the PE clock after ~3.4us of sustained
            # activity; spin on a zeroed tile while the first DMAs land.
            wz = cpool.tile([P, RB], bf16, tag="wz", name="wz")
            nc.vector.memset(wz, 0.0)
            spin = pspool.tile([P, RB], f32, tag="py", name="spin", bufs=2)
            for _ in range(5):
                nc.tensor.matmul(spin, lhsT=wz[:, :P], rhs=wz, start=True, stop=True)

            for rb in range(NRB):
                cols = slice(rb * RB, (rb + 1) * RB)

                sig_t = {}

                def emit_po(m, rb=rb, cols=cols, sig_t=sig_t):
                    msl = slice(m * P, (m + 1) * P)
                    po = pspool.tile([P, RB], f32, tag="po", name=f"po{rb}_{m}", bufs=3)
                    for kk in range(KK):
                        nc.tensor.matmul(
                            po,
                            lhsT=wo8S[:, kk, :, msl],
                            rhs=x8S[:, rb, kk],
                            start=(kk == 0),
                            stop=(kk == KK - 1),
                            perf_mode=DR,
                        )
                    sig = opool.tile([P, RB], bf16, tag="sig", name=f"sig{rb}_{m}")
                    nc.scalar.activation(
                        out=sig, in_=po,
                        func=mybir.ActivationFunctionType.Sigmoid,
                        bias=biasS[:, 1, m : m + 1], scale=1.0 / WO_SCALE,
                    )
                    sig_t[m] = sig

                # o-gate two m-tiles ahead: its fp8 weights+x arrive first, so
                # the PE has work while the bf16 weights are still loading.
                emit_po(0)
                emit_po(1)
                gS = [
                    wpool.tile([P, RB], bf16, tag=f"g{m}", name=f"gS{rb}_{m}")
                    for m in range(KT)
                ]
                for m in range(KT):
                    msl = slice(m * P, (m + 1) * P)
                    ph = pspool.tile([P, RB], f32, tag="ph", name=f"ph{rb}_{m}", bufs=3)
                    for k in range(KT):
                        nc.tensor.matmul(
                            ph, lhsT=wgS[:, k, msl], rhs=xbS[:, k, cols],
                            start=(k == 0), stop=(k == KT - 1),
                        )
                    if m + 2 < KT:
                        emit_po(m + 2)
                    nc.vector.scalar_tensor_tensor(
                        out=gS[m], in0=ph, scalar=biasS[:, 0, m : m + 1],
                        in1=sig_t[m],
                        op0=mybir.AluOpType.add, op1=mybir.AluOpType.mult,
                    )
                # ---- layer 2: yT = gT.T-contract @ W_out (+ b_out)
                for n in range(KT):
                    nsl = slice(n * P, (n + 1) * P)
                    py = pspool.tile([P, RB], f32, tag="py", name=f"py{rb}_{n}", bufs=2)
                    for m in range(KT):
                        nc.tensor.matmul(
                            py, lhsT=wuS[:, m, nsl], rhs=gS[m],
                            start=(m == 0), stop=(m == KT - 1),
                        )
                    yo = opool.tile([P, RB], bf16, tag="yo", name=f"yo{rb}_{n}")
                    # bias-add on DVE, keeping ScalarE free for sigmoids
                    nc.vector.tensor_scalar_add(yo, py, biasS[:, 2, n : n + 1])
                    nc.sync.dma_start(out=yT[nsl, cols], in_=yo)
    nc.finalize()
    return nc


def kernel(x, W_proj, b_proj, W_out, b_out, layer_idx=0, num_layers=12):
    global _NC, LAST_RESULT
    x = np.asarray(x, dtype=np.float32)
    W_proj = np.asarray(W_proj, dtype=np.float32)
    b_proj = np.asarray(b_proj, dtype=np.float32)
    W_out = np.asarray(W_out, dtype=np.float32)
    b_out = np.asarray(b_out, dtype=np.float32)

    wg_f = W_proj[:, :D]
    wo_f = W_proj[:, 2 * D : 3 * D]
    # k-tile-major weight layouts
    wg_h = np.ascontiguousarray(wg_f.reshape(KT, P, D)).astype(BF16)
    wu_h = np.ascontiguousarray(W_out.reshape(KT, P, D)).astype(BF16)
    wo8_h = np.ascontiguousarray(
        (wo_f * WO_SCALE).reshape(KK, 2, P, D).transpose(0, 2, 1, 3)
    ).astype(F8)
    bias_h = np.ascontiguousarray(
        np.stack(
            [
                b_proj[:D].reshape(KT, P).T,
                b_proj[2 * D : 3 * D].reshape(KT, P).T,
                b_out.reshape(KT, P).T,
            ]
        )
    )

    xf = x.reshape(ROWS, D)
    in_maps = []
    for c in range(NCORES):
        xT = np.ascontiguousarray(xf[c * RPC : (c + 1) * RPC, :].T)  # [D, RPC]
        xb_h = xT.reshape(KT, P, RPC).astype(BF16)
        # [NRB, KK, P, 2, RB]: block-major fp8 pairs for DoubleRow
        x8_h = np.ascontiguousarray(
            xT.reshape(KK, 2, P, NRB, RB).transpose(3, 0, 2, 1, 4)
        ).astype(F8)
        in_maps.append(
            {"xb": xb_h, "x8": x8_h, "wg": wg_h, "wu": wu_h,
             "wo8": wo8_h, "bias": bias_h}
        )

    if _NC is None:
        _NC = _build()

    trace = os.environ.get("HGRN_TRACE", "0") == "1"
    LAST_RESULT = run_bass_kernel_spmd(
        _NC, in_maps, core_ids=list(range(NCORES)), trace=trace,
        tmpdir=os.environ.get("HGRN_TMPDIR"),
    )
    y = np.empty((ROWS, D), dtype=np.float32)
    for c in range(NCORES):
        y[c * RPC : (c + 1) * RPC, :] = np.asarray(
            LAST_RESULT.results[c]["yT"], dtype=np.float32
        ).T
    return y.reshape(B, L, D)
